# revision 7
# baseline (speedup 1.0000x reference)
"""DSAttention Trainium2 kernel.

Reference math (per batch b, head h):
    scores = (Q @ K^T) * tau[b] + delta[b][key]        # [L, S]
    scores = causal_mask(scores)                        # -inf above diagonal
    attn   = softmax(scale * scores)                    # scale = 1/sqrt(D)
    out    = attn @ V

Sharding: batch -> core (B == n_cores == 8); each core computes all 16 heads
of its batch. No cross-core communication.

Device kernel (per head; L=1024, D=64, P=128, NB=8 s-chunks):
 - Host folds scale*tau into K (so the score matmul output needs no scaling)
   and pre-transposes: per head-pair, kq2 = [K^T(h0);K^T(h1) | Q^T(h0);Q^T(h1)]
   packed [128, 2048] for full-bandwidth DMA. The per-key bias scale*delta
   rides as the ScalarE activation bias operand (per-partition [128,1]).
 - S^T[s,l] per s-chunk i: matmul(lhsT=K^T chunk [64,128], rhs=Q^T [64,<=512])
   in float32r (1 cycle/row on the PE at N>=256; fp32 costs 4).
   Causal block sparsity: only l >= i*128 computed; pieces narrower than 256
   columns are widened leftward (the widened region is zeroed in E^T).
 - E^T = exp(S^T + delta_bias) in one ACT per chunk; diagonal 128x128 block
   masked in-place on GpSimd via affine_select (keep l >= s else 0).
 - O^T[65,1024] accumulates in PSUM: lhsT = [V|1] chunk [128,65] stationary,
   rhs = E^T moving (f32r). Row 64 = softmax denominator via the ones column.
 - Epilogue in 2 groups of 4 l-chunks: PE-transpose [65,128] -> [128,4,65]
   PSUM, one reciprocal [128,4], one broadcast tensor_tensor multiply
   [128,4,64], one DMA per group to the natural [L, D] output layout.

Softmax without max-subtraction is exact softmax math; these inputs keep
|scale*scores| < ~10 so exp stays far inside fp32 range.
"""

import sys

if "/opt/trn_rl_repo" not in sys.path:
    sys.path.insert(0, "/opt/trn_rl_repo")

import numpy as np

from concourse import bacc, mybir, tile
import concourse.bass as bass
from concourse.bass_utils import run_bass_kernel_spmd

B, L, H, D = 8, 1024, 16, 64
P = 128          # partition tile
NB = L // P      # 8 s-chunks
DP = D + 1       # 65: head dim + ones column in V
SCALE = 1.0 / float(np.sqrt(D))
F32 = mybir.dt.float32
F32R = mybir.dt.float32r
BF16 = mybir.dt.bfloat16
_DT_BY_NAME = {"f32": F32, "f32r": F32R, "bf16": BF16}
import os as _os
ST_DT = _DT_BY_NAME[_os.environ.get("KERNEL_ST_DT", "f32r")]
AV_DT = _DT_BY_NAME[_os.environ.get("KERNEL_AV_DT", "f32r")]
N_CORES = 8
MIN_N = 256      # min f32r moving-dim for 1 cycle/row


def _pieces(i, causal):
    """[(ext_lo, true_lo, end), ...] column pieces for s-chunk i.

    ext_lo..true_lo is dead (masked) region computed only to keep the
    matmul moving-dim >= MIN_N; it is zeroed in E^T before the AV matmul.
    """
    if not causal:
        return [(0, 0, 512), (512, 512, 1024)]
    lo = i * P
    out = []
    if lo < 512:
        out.append((min(lo, 512 - MIN_N), lo, 512))
        out.append((512, 512, 1024))
    else:
        out.append((min(lo, 1024 - MIN_N), lo, 1024))
    return out


def _build(n_heads=H, causal=True, st_dt=None, av_dt=None):
    st_dt = ST_DT if st_dt is None else st_dt
    av_dt = AV_DT if av_dt is None else av_dt
    nc = bacc.Bacc("TRN2", target_bir_lowering=False, debug=False)
    n_pairs = (n_heads + 1) // 2

    # [pair, {k,q}, 128, L]: partitions 0:64 = even head, 64:128 = odd head
    kq2 = nc.dram_tensor("kq2", [n_pairs, 2, P, L], st_dt, kind="ExternalInput")
    vo = nc.dram_tensor("vo", [n_heads, P, NB, DP], av_dt, kind="ExternalInput")
    deltas = nc.dram_tensor("deltas", [P, NB], F32, kind="ExternalInput")
    otn = nc.dram_tensor("otn", [n_heads, D, L], F32, kind="ExternalOutput")

    half_contrib = [[], []]
    for i in range(NB):
        for (el, _, en) in _pieces(i, causal):
            half_contrib[0 if el < 512 else 1].append(i)

    with tile.TileContext(nc) as tc:
        with (
            tc.tile_pool(name="const", bufs=1) as cpool,
            tc.tile_pool(name="kq", bufs=2) as kqpool,
            tc.tile_pool(name="vpool", bufs=2) as vpool,
            tc.tile_pool(name="et", bufs=3) as etpool,
            tc.tile_pool(name="rec", bufs=2) as recpool,
            tc.tile_pool(name="bcast", bufs=2) as bcpool,
            tc.tile_pool(name="fin", bufs=2) as finpool,
            tc.tile_pool(name="st_ps", bufs=2, space=bass.MemorySpace.PSUM) as stps,
            tc.tile_pool(name="o_ps", bufs=2, space=bass.MemorySpace.PSUM) as ops,
        ):
            delta_sb = cpool.tile([P, NB], F32, tag="deltas")
            nc.sync.dma_start(delta_sb[:], deltas[:])
            zero_reg = nc.gpsimd.to_reg(0.0)

            for h in range(n_heads):
                if h % 2 == 0:
                    kq_sb = kqpool.tile([P, 2, L], st_dt, tag="kq",
                                        name=f"kq_sb{h}")
                    nc.sync.dma_start(
                        kq_sb[:], kq2[h // 2].rearrange("t p l -> p t l")
                    )
                base = 64 * (h % 2)
                ksb = kq_sb[base:base + 64, 0, :]
                qsb = kq_sb[base:base + 64, 1, :]

                v_sb = vpool.tile([P, NB, DP], av_dt, tag="v", name=f"v_sb{h}")
                nc.sync.dma_start(v_sb[:], vo[h])

                o_half = [
                    ops.tile([DP, 512], F32, tag="o0", name=f"o0_h{h}"),
                    ops.tile([DP, 512], F32, tag="o1", name=f"o1_h{h}"),
                ]

                for i in range(NB):
                    et = etpool.tile([P, L], av_dt, tag="et", name=f"et{h}_{i}")
                    st = stps.tile([P, L], F32, tag="st", name=f"st{h}_{i}")
                    pieces = _pieces(i, causal)
                    for (el, _, en) in pieces:
                        nc.tensor.matmul(
                            st[:, el:en],
                            ksb[:, i * P:(i + 1) * P],
                            qsb[:, el:en],
                            start=True,
                            stop=True,
                        )
                    # one exp over the whole computed range (including the
                    # widened dead region, which affine_select zeroes next —
                    # exp'ing it keeps every byte the mask reads initialized);
                    # bias = scale*delta[s]
                    act_lo = pieces[0][0]
                    nc.scalar.activation(
                        et[:, act_lo:L], st[:, act_lo:L],
                        mybir.ActivationFunctionType.Exp,
                        bias=delta_sb[:, i:i + 1],
                    )
                    if causal:
                        # zero everything left of the diagonal in one op:
                        # covers the widened dead region [ext_lo, dc) plus the
                        # triangular part of the diag block [dc, dc+P).
                        # keep where l >= s: l = ext_lo + y, s = dc + x.
                        ext_lo = pieces[0][0]
                        dc = i * P
                        w = dc + P - ext_lo
                        nc.gpsimd.affine_select(
                            out=et[:, ext_lo:dc + P],
                            in_=et[:, ext_lo:dc + P],
                            compare_op=mybir.AluOpType.is_ge,
                            fill=zero_reg,
                            base=ext_lo - dc,
                            pattern=[[1, w]],
                            channel_multiplier=-1,
                        )
                    for (el, _, en) in pieces:
                        hi = 0 if el < 512 else 1
                        contrib = half_contrib[hi]
                        nc.tensor.matmul(
                            o_half[hi][:, el - hi * 512: en - hi * 512],
                            v_sb[:, i, :],
                            et[:, el:en],
                            start=(i == contrib[0]),
                            stop=(i == contrib[-1]),
                        )

                # normalize in O^T layout: rec = 1/denom row, broadcast
                # across the 64 head-dim partitions on GpSimd, multiply.
                rec = recpool.tile([1, L], F32, tag="rec", name=f"rec{h}")
                for g in range(2):
                    nc.vector.reciprocal(
                        rec[:, g * 512:(g + 1) * 512], o_half[g][D:DP, :]
                    )
                bc = bcpool.tile([D, L], F32, tag="bc", name=f"bc{h}")
                nc.gpsimd.partition_broadcast(bc[:], rec[:], channels=D)
                fin = finpool.tile([D, L], F32, tag="fin", name=f"fin{h}")
                for g in range(2):
                    nc.vector.tensor_tensor(
                        fin[:, g * 512:(g + 1) * 512], o_half[g][0:D, :],
                        bc[:, g * 512:(g + 1) * 512],
                        mybir.AluOpType.mult,
                    )
                nc.sync.dma_start(otn[h], fin[:])

    nc.compile()
    return nc


_PROGRAMS = {}


def _get_program(causal):
    key = (causal,)
    if key not in _PROGRAMS:
        _PROGRAMS[key] = _build(H, causal)
    return _PROGRAMS[key]


_CAUSAL_MASK = None


def _mask_kind(attn_mask):
    """'causal' | 'none' | 'other' for the given [B,1,L,L] bool mask."""
    global _CAUSAL_MASK
    m = np.asarray(attn_mask)
    if not m.any():
        return "none"
    if _CAUSAL_MASK is None:
        _CAUSAL_MASK = np.triu(np.ones((L, L), dtype=bool), k=1)
    if m.shape == (B, 1, L, L) and all(
        np.array_equal(m[b, 0], _CAUSAL_MASK) for b in range(B)
    ):
        return "causal"
    return "other"


def _prep_core_inputs(queries, keys, values, tau, delta):
    """Build per-core input maps (host-side shard + layout prep)."""
    deltas_all = (np.float32(SCALE) * delta.astype(np.float32)).reshape(B, NB, P)
    in_maps = []
    for b in range(B):
        a = np.float32(SCALE) * np.float32(tau[b, 0])
        # kq2[pair, 0] = scaled K^T of (even; odd) heads, kq2[pair, 1] = Q^T
        kt = keys[b].transpose(1, 2, 0).astype(np.float32) * a    # [H, D, L]
        qt = queries[b].transpose(1, 2, 0).astype(np.float32)     # [H, D, L]
        kq = np.stack([kt.reshape(H // 2, P, L),
                       qt.reshape(H // 2, P, L)], axis=1)         # [H/2,2,P,L]
        v = values[b].astype(np.float32)                          # [L, H, D]
        voh = np.empty((H, P, NB, DP), dtype=np.float32)
        voh[..., D] = 1.0
        # v [L,H,D] -> [H, NB, P, D] -> [H, P, NB, D]
        voh[..., :D] = v.transpose(1, 0, 2).reshape(H, NB, P, D).transpose(0, 2, 1, 3)
        in_maps.append({
            "kq2": np.ascontiguousarray(kq).astype(mybir.dt.np(ST_DT)),
            "vo": voh.astype(mybir.dt.np(AV_DT)),
            "deltas": np.ascontiguousarray(deltas_all[b].T),  # [P, NB]
        })
    return in_maps


def _assemble(results):
    """Per-core [H, L, D] -> full [B, L, H, D]."""
    outs = [np.asarray(r["otn"]).transpose(2, 0, 1) for r in results]
    return np.ascontiguousarray(np.stack(outs, axis=0))


def _run(inputs, trace=False):
    queries = np.asarray(inputs["queries"], dtype=np.float32)
    keys = np.asarray(inputs["keys"], dtype=np.float32)
    values = np.asarray(inputs["values"], dtype=np.float32)
    tau = np.asarray(inputs["tau"], dtype=np.float32)
    delta = np.asarray(inputs["delta"], dtype=np.float32)
    kind = _mask_kind(inputs["attn_mask"])
    if kind == "other":
        # Arbitrary masks are outside this kernel's fast path; fall back to a
        # correct host computation.
        m = np.asarray(inputs["attn_mask"])
        scores = np.einsum("blhe,bshe->bhls", queries, keys)
        scores = scores * tau[:, None, None, :] + delta[:, None, None, :]
        scores = np.where(m, -np.inf, scores) * SCALE
        scores -= scores.max(axis=-1, keepdims=True)
        e = np.exp(scores)
        attn = e / e.sum(axis=-1, keepdims=True)
        return np.einsum("bhls,bshd->blhd", attn, values).astype(np.float32), None

    nc = _get_program(causal=(kind == "causal"))
    in_maps = _prep_core_inputs(queries, keys, values, tau, delta)
    res = run_bass_kernel_spmd(
        nc, in_maps, core_ids=list(range(N_CORES)), trace=trace
    )
    return _assemble(res.results), res


def kernel(**inputs):
    out, _ = _run(inputs, trace=False)
    return out


def kernel_traced(**inputs):
    """Like kernel(), but also returns the BassKernelResults (exec_time_ns)."""
    out, res = _run(inputs, trace=True)
    return out, res
